# revision 53
# baseline (speedup 1.0000x reference)
"""Category-specific MLP (MoE-style routing) for Trainium2, 8 NeuronCores.

Reference computation (per token n):
    h   = relu(x[n] @ W1[cat[n]] + b1[cat[n]])      x:[N,128]  W1:[100,128,128]
    out = h @ W2[cat[n]] + b2[cat[n]]               W2:[100,128,64]

Strategy (expert-parallel, MoE-style):
  * Host: sort tokens by category. Split any category with more than 512
    tokens into work items of <=512 tokens. Sort items by size (desc) and
    assign item of rank r to (core r%8, slot r//8). All cores run the same
    SPMD program with S slots; slot s has fixed column capacity caps[s] =
    size of the largest item in that slot across cores, so the instruction
    stream and shapes are identical on every core while padding stays
    minimal (~5% for the target distribution).
  * Everything is kept feature-on-partitions (transposed). Slots are packed
    into PSUM groups of <=512 total columns (one PSUM bank per group-layer).
    The per-core fp16 blob holds two row-major [128, W] halves (groups side
    by side, per group: w1 ns*128 | w2 ns*64 | xT cols); each half is one
    big-line DMA on its own HWDGE ring (SP / ACT) so both stream
    concurrently at ~2x single-ring rate.
    Per slot (fp16 matmuls, fp32 PSUM accumulate):
        psum1[:, lo:lo+B] = W1_s^T @ xT_s        (PE)
        psum2[:, lo:lo+B] = W2_s^T @ h_s         (PE)
    Per group (merged PSUM evacuation on DVE, valid because b1/b2 == 0;
    a per-slot bias path is emitted instead when biases are nonzero):
        h_g   = max(psum1_g, 0)   -> fp16 SBUF
        out_g = copy(psum2_g)     -> fp32 SBUF -> DMA
    Groups are software-pipelined (layer-2 of group g emitted after layer-1
    of group g+1) so the PE stream never waits on an evacuation.
  * Host: scatter outT columns back to the original token order.

fp16 numerics: inputs are rounded to fp16 (10-bit mantissa), accumulation
is fp32 in PSUM. Measured vs the fp32 reference: resid_var ~2e-7,
absmax-relative error ~5e-4.
"""

from contextlib import ExitStack

import numpy as np

import concourse.bass as bass
import concourse.mybir as mybir
import concourse.tile as tile
from concourse import bacc
from concourse.bass_utils import run_bass_kernel_spmd

N, C, D, H, O = 8192, 100, 128, 128, 64
NCORES = 8
MAX_ITEM = 512      # PSUM bank / moving-operand limit (fp32 columns)
GROUP_COLS = 512    # column budget per group (one PSUM bank, fp32)

F16 = mybir.dt.float16
F32 = mybir.dt.float32


def _plan(cat_ids: np.ndarray, zero_bias: bool):
    """Host-side routing plan: work items -> (core, slot) assignment."""
    cat_ids = np.asarray(cat_ids).astype(np.int64)
    counts = np.bincount(cat_ids, minlength=C)
    NC = len(counts)                                    # robust to ids >= C
    order = np.argsort(cat_ids, kind="stable")          # token ids sorted by cat
    starts = np.zeros(NC, dtype=np.int64)
    starts[1:] = np.cumsum(counts)[:-1]

    items = []                                          # (cat, start_in_cat, len)
    for c in range(NC):
        cnt = int(counts[c])
        o = 0
        while o < cnt:
            ln = min(MAX_ITEM, cnt - o)
            items.append((c, o, ln))
            o += ln
    items.sort(key=lambda it: -it[2])

    S = (len(items) + NCORES - 1) // NCORES
    grid = [[None] * NCORES for _ in range(S)]          # grid[s][k] = item|None
    for r, it in enumerate(items):
        grid[r // NCORES][r % NCORES] = it
    caps = tuple(max(1, max((it[2] for it in row if it is not None), default=1))
                 for row in grid)
    offs = np.zeros(S + 1, dtype=np.int64)
    offs[1:] = np.cumsum(caps)
    T = int(offs[-1])

    # Split slots into two load halves balanced by bytes (half A slightly
    # lighter so the PE starts on it while half B is still streaming), then
    # pack each half's slots into PSUM groups of <= GROUP_COLS columns.
    slot_bytes = [(H + O + int(caps[s])) * 256 for s in range(S)]
    total_b = sum(slot_bytes)
    acc, s_split = 0, S
    for s in range(S):
        acc += slot_bytes[s]
        if acc >= total_b * 0.39:
            s_split = s + 1
            break
    s_split = max(1, min(s_split, S))

    groups = []
    for (lo, hi) in ((0, s_split), (s_split, S)):
        s0 = lo
        while s0 < hi:
            s1 = s0 + 1
            while s1 < hi and int(offs[s1 + 1] - offs[s0]) <= GROUP_COLS:
                s1 += 1
            groups.append((s0, s1))
            s0 = s1
    n_a = sum(1 for (s0, s1) in groups if s1 <= s_split)

    # The blob is loaded as two halves, one DMA per HWDGE ring (SP + ACT),
    # each a [128, W_half] row-major block with its groups side by side as
    # column ranges (columns per group: w1 ns*H | w2 ns*O | x cols).
    # Half A carries the first and last groups (processed 1st and 2nd),
    # half B the middle — both rings stream concurrently and the PE starts
    # on half A while half B is still arriving.
    G = len(groups)
    halves = [list(range(n_a)), list(range(n_a, G))]
    proc = halves[0] + halves[1]            # PE processing order

    def gwidth(gi):
        s0, s1 = groups[gi]
        return (s1 - s0) * (H + O) + int(offs[s1] - offs[s0])

    gbase = {}                              # gi -> (half, col base in half)
    half_w = []
    for hi, gis in enumerate(halves):
        w = 0
        for gi in gis:
            gbase[gi] = (hi, w)
            w += gwidth(gi)
        half_w.append(w)
    Z = 128 * (half_w[0] + half_w[1])

    return {
        "order": order, "starts": starts, "grid": grid,
        "S": S, "caps": caps, "offs": offs, "T": T,
        "groups": groups, "Z": Z, "zero_bias": zero_bias,
        "halves": halves, "proc": proc, "gbase": gbase, "half_w": half_w,
    }


_NC_CACHE: dict = {}


def _build_nc(plan):
    S, caps, T, Z = plan["S"], plan["caps"], plan["T"], plan["Z"]
    zero_bias = plan["zero_bias"]
    key = (S, caps, zero_bias)
    if key in _NC_CACHE:
        return _NC_CACHE[key]

    offs, groups = plan["offs"], plan["groups"]
    gbase, half_w, proc = plan["gbase"], plan["half_w"], plan["proc"]
    G = len(groups)

    nc = bacc.Bacc("TRN2", target_bir_lowering=False, debug=False,
                   enable_partition_id=False)
    blob_d = nc.dram_tensor("blob", [Z], F16, kind="ExternalInput").ap()
    if not zero_bias:
        bias_d = nc.dram_tensor("bias", [128, 2 * S], F32,
                                kind="ExternalInput").ap()
    out_d = nc.dram_tensor("outT", [O * T], F16, kind="ExternalOutput").ap()

    # Two halves, one DMA per HWDGE ring (SP + ACT): the SDMA engines drain
    # both rings' packets concurrently, so the two big transfers (3-4KB
    # per-partition lines) sustain ~2x the single-ring rate; fine-grained
    # staggered loads can't beat this because SDMA round-robin is
    # packet-fair and equalizes completion times anyway.
    with tile.TileContext(nc) as tc, ExitStack() as ctx:
        loads = ctx.enter_context(tc.tile_pool(name="loads", bufs=1))
        hbuf = ctx.enter_context(tc.tile_pool(name="hbuf", bufs=3))
        obuf = ctx.enter_context(tc.tile_pool(name="obuf", bufs=3))
        ps1p = ctx.enter_context(tc.tile_pool(name="ps1p", bufs=2, space="PSUM"))
        ps2p = ctx.enter_context(tc.tile_pool(name="ps2p", bufs=2, space="PSUM"))

        WA, WB = half_w[0], half_w[1]
        blk_a = loads.tile([128, WA], F16)
        nc.sync.dma_start(out=blk_a,
                          in_=blob_d[0:128 * WA].rearrange("(p w) -> p w", p=128))
        if WB:
            blk_b = loads.tile([128, WB], F16)
            nc.scalar.dma_start(
                out=blk_b,
                in_=blob_d[128 * WA:Z].rearrange("(p w) -> p w", p=128))

        def group_view(gi):
            hi, base = gbase[gi]
            return (blk_a if hi == 0 else blk_b), base

        # PE is idle for ~4us while the blob streams in; HAM keeps a cold PE
        # at 1.2GHz until it has seen ~3.4us of sustained activity. Burn the
        # DMA wait with dummy matmuls on a zeroed tile so the real matmuls
        # run warm at 2.4GHz. Alternating two PSUM banks keeps them
        # back-to-back (~427ns each); they must drain before the first load
        # lands so the real stream is never queued behind them.
        warm = ctx.enter_context(tc.tile_pool(name="warm", bufs=1))
        wz = warm.tile([128, 512], F16)
        nc.vector.memset(wz, 0.0)
        wps = ctx.enter_context(
            tc.tile_pool(name="wps", bufs=1, space="PSUM"))
        wp0 = wps.tile([128, 512], F32, name="wp0")
        wp1 = wps.tile([128, 512], F32, name="wp1")
        for j in range(6):
            nc.tensor.matmul(wp0 if j % 2 == 0 else wp1,
                             lhsT=wz[:, 0:128], rhs=wz,
                             start=True, stop=True)
        if not zero_bias:
            consts = ctx.enter_context(tc.tile_pool(name="consts", bufs=1))
            bias = consts.tile([128, 2 * S], F32)
            nc.sync.dma_start(out=bias, in_=bias_d)

        state = {}      # per live group: tiles needed by the layer-2 phase

        def phase1(gi):
            s0, s1 = groups[gi]
            ns = s1 - s0
            co0, co1 = int(offs[s0]), int(offs[s1])
            cols = co1 - co0
            blk, base = group_view(gi)
            xv_base = base + ns * (H + O)
            ps1 = ps1p.tile([H, cols], F32, tag="ps1", name=f"ps1_{gi}")
            for s in range(s0, s1):
                i, B = s - s0, int(caps[s])
                lo = int(offs[s]) - co0
                nc.tensor.matmul(ps1[:, lo:lo + B],
                                 lhsT=blk[:, base + i * H:base + (i + 1) * H],
                                 rhs=blk[:, xv_base + lo:xv_base + lo + B],
                                 start=True, stop=True)
            h_g = hbuf.tile([H, cols], F16, tag="h", name=f"h_{gi}")
            if zero_bias:
                if ns >= 4:
                    # split the evacuation at a slot boundary so layer-2
                    # matmuls of the first half start ~300ns earlier
                    sm = s0 + ns // 2
                    mid = int(offs[sm]) - co0
                    nc.vector.tensor_scalar_max(h_g[:, 0:mid],
                                                ps1[:, 0:mid], 0.0)
                    nc.vector.tensor_scalar_max(h_g[:, mid:cols],
                                                ps1[:, mid:cols], 0.0)
                else:
                    nc.vector.tensor_scalar_max(h_g, ps1, 0.0)
            else:
                for s in range(s0, s1):
                    i, B = s - s0, int(caps[s])
                    lo = int(offs[s]) - co0
                    nc.vector.tensor_scalar(
                        h_g[:, lo:lo + B], ps1[:, lo:lo + B], bias[:, s:s + 1],
                        0.0, mybir.AluOpType.add, mybir.AluOpType.max)
            state[gi] = h_g

        def phase2(gi):
            s0, s1 = groups[gi]
            ns = s1 - s0
            co0, co1 = int(offs[s0]), int(offs[s1])
            cols = co1 - co0
            h_g = state.pop(gi)
            blk, base = group_view(gi)
            w2_base = base + ns * H
            ps2 = ps2p.tile([O, cols], F32, tag="ps2", name=f"ps2_{gi}")
            for s in range(s0, s1):
                i, B = s - s0, int(caps[s])
                lo = int(offs[s]) - co0
                nc.tensor.matmul(ps2[:, lo:lo + B],
                                 lhsT=blk[:, w2_base + i * O:w2_base + (i + 1) * O],
                                 rhs=h_g[:, lo:lo + B], start=True, stop=True)
            o_g = obuf.tile([O, cols], F16, tag="o", name=f"o_{gi}")
            if zero_bias:
                nc.vector.tensor_copy(o_g, ps2)
            else:
                for s in range(s0, s1):
                    i, B = s - s0, int(caps[s])
                    lo = int(offs[s]) - co0
                    nc.vector.tensor_scalar_add(o_g[:, lo:lo + B],
                                                ps2[:, lo:lo + B],
                                                bias[0:O, S + s:S + s + 1])
            dst = out_d[O * co0:O * co1].rearrange("(p w) -> p w", p=O)
            nc.scalar.dma_start(out=dst, in_=o_g)

        # software pipeline: layer-2 of group g rides behind layer-1 of the
        # next group in processing order
        phase1(proc[0])
        for i in range(1, G):
            phase1(proc[i])
            phase2(proc[i - 1])
        phase2(proc[G - 1])

    nc.compile()
    _NC_CACHE[key] = nc
    return nc


def _shard_inputs(x, W1, b1, W2, b2, plan):
    S, offs, Z = plan["S"], plan["offs"], plan["Z"]
    order, starts, grid = plan["order"], plan["starts"], plan["grid"]
    groups = plan["groups"]
    gbase, half_w = plan["gbase"], plan["half_w"]

    x16 = x.astype(np.float16)
    W116 = W1.astype(np.float16)
    W216 = W2.astype(np.float16)

    split = 128 * half_w[0]
    in_maps = []
    for k in range(NCORES):
        blob = np.zeros(Z, dtype=np.float16)
        blks = [blob[0:split].reshape(128, half_w[0]),
                blob[split:Z].reshape(128, half_w[1]) if half_w[1] else None]
        if not plan["zero_bias"]:
            biasc = np.zeros((128, 2 * S), dtype=np.float32)
        for gi, (s0, s1) in enumerate(groups):
            ns = s1 - s0
            co0 = int(offs[s0])
            cols = int(offs[s1]) - co0
            W_g = ns * (H + O) + cols
            hi, base = gbase[gi]
            gb = blks[hi][:, base:base + W_g]
            for s in range(s0, s1):
                it = grid[s][k]
                if it is None:
                    continue
                i = s - s0
                c, o, ln = it
                toks = order[starts[c] + o: starts[c] + o + ln]
                gb[:, i * H:(i + 1) * H] = W116[c]
                gb[:, ns * H + i * O:ns * H + (i + 1) * O] = W216[c]
                xoff = ns * (H + O) + (int(offs[s]) - co0)
                gb[:, xoff:xoff + ln] = x16[toks].T
                if not plan["zero_bias"]:
                    biasc[:, s] = b1[c]
                    biasc[0:O, S + s] = b2[c]
        m = {"blob": blob}
        if not plan["zero_bias"]:
            m["bias"] = biasc
        in_maps.append(m)
    return in_maps


def _unshard(results, plan):
    S, offs, T = plan["S"], plan["offs"], plan["T"]
    order, starts, grid = plan["order"], plan["starts"], plan["grid"]
    groups = plan["groups"]
    out = np.empty((N, O), dtype=np.float32)
    for k in range(NCORES):
        flat = results[k]["outT"].astype(np.float32)
        for (s0, s1) in groups:
            co0, co1 = int(offs[s0]), int(offs[s1])
            blk = flat[O * co0:O * co1].reshape(O, co1 - co0)
            for s in range(s0, s1):
                it = grid[s][k]
                if it is None:
                    continue
                c, o, ln = it
                toks = order[starts[c] + o: starts[c] + o + ln]
                lo = int(offs[s]) - co0
                out[toks] = blk[:, lo:lo + ln].T
    return out


def _execute(x, cat_ids, W1, b1, W2, b2, trace=False):
    x = np.asarray(x, dtype=np.float32)
    W1 = np.asarray(W1, dtype=np.float32)
    b1 = np.asarray(b1, dtype=np.float32)
    W2 = np.asarray(W2, dtype=np.float32)
    b2 = np.asarray(b2, dtype=np.float32)

    zero_bias = not (b1.any() or b2.any())
    plan = _plan(cat_ids, zero_bias)
    nc = _build_nc(plan)
    in_maps = _shard_inputs(x, W1, b1, W2, b2, plan)
    res = run_bass_kernel_spmd(nc, in_maps, list(range(NCORES)), trace=trace)
    out = _unshard(res.results, plan)
    return out, res


def kernel(x, cat_ids, W1, b1, W2, b2):
    out, _ = _execute(x, cat_ids, W1, b1, W2, b2, trace=False)
    return out


# revision 56
# speedup vs baseline: 1.0306x; 1.0306x over previous
"""Category-specific MLP (MoE-style routing) for Trainium2, 8 NeuronCores.

Reference computation (per token n):
    h   = relu(x[n] @ W1[cat[n]] + b1[cat[n]])      x:[N,128]  W1:[100,128,128]
    out = h @ W2[cat[n]] + b2[cat[n]]               W2:[100,128,64]

Strategy (expert-parallel, MoE-style):
  * Host: sort tokens by category. Split any category with more than 512
    tokens into work items of <=512 tokens. Sort items by size (desc) and
    assign item of rank r to (core r%8, slot r//8). All cores run the same
    SPMD program with S slots; slot s has fixed column capacity caps[s] =
    size of the largest item in that slot across cores, so the instruction
    stream and shapes are identical on every core while padding stays
    minimal (~5% for the target distribution).
  * Everything is kept feature-on-partitions (transposed). Slots are packed
    into PSUM groups of <=512 total columns (one PSUM bank per group-layer).
    The per-core fp16 blob holds two row-major [128, W] halves (groups side
    by side, per group: w1 ns*128 | w2 ns*64 | xT cols); each half is one
    big-line DMA on its own HWDGE ring (SP / ACT) so both stream
    concurrently at ~2x single-ring rate.
    Per slot (fp16 matmuls, fp32 PSUM accumulate):
        psum1[:, lo:lo+B] = W1_s^T @ xT_s        (PE)
        psum2[:, lo:lo+B] = W2_s^T @ h_s         (PE)
    Per group (merged PSUM evacuation on DVE, valid because b1/b2 == 0;
    a per-slot bias path is emitted instead when biases are nonzero):
        h_g   = max(psum1_g, 0)   -> fp16 SBUF
        out_g = copy(psum2_g)     -> fp32 SBUF -> DMA
    Groups are software-pipelined (layer-2 of group g emitted after layer-1
    of group g+1) so the PE stream never waits on an evacuation.
  * Host: scatter outT columns back to the original token order.

fp16 numerics: inputs are rounded to fp16 (10-bit mantissa), accumulation
is fp32 in PSUM. Measured vs the fp32 reference: resid_var ~2e-7,
absmax-relative error ~5e-4.
"""

from contextlib import ExitStack

import numpy as np

import concourse.bass as bass
import concourse.mybir as mybir
import concourse.tile as tile
from concourse import bacc
from concourse.bass_utils import run_bass_kernel_spmd

N, C, D, H, O = 8192, 100, 128, 128, 64
NCORES = 8
MAX_ITEM = 512      # PSUM bank / moving-operand limit (fp32 columns)
GROUP_COLS = 512    # column budget per group (one PSUM bank, fp32)

F16 = mybir.dt.float16
F32 = mybir.dt.float32


def _plan(cat_ids: np.ndarray, zero_bias: bool):
    """Host-side routing plan: work items -> (core, slot) assignment."""
    cat_ids = np.asarray(cat_ids).astype(np.int64)
    counts = np.bincount(cat_ids, minlength=C)
    NC = len(counts)                                    # robust to ids >= C
    order = np.argsort(cat_ids, kind="stable")          # token ids sorted by cat
    starts = np.zeros(NC, dtype=np.int64)
    starts[1:] = np.cumsum(counts)[:-1]

    items = []                                          # (cat, start_in_cat, len)
    for c in range(NC):
        cnt = int(counts[c])
        o = 0
        while o < cnt:
            ln = min(MAX_ITEM, cnt - o)
            items.append((c, o, ln))
            o += ln
    items.sort(key=lambda it: -it[2])

    S = (len(items) + NCORES - 1) // NCORES
    grid = [[None] * NCORES for _ in range(S)]          # grid[s][k] = item|None
    for r, it in enumerate(items):
        grid[r // NCORES][r % NCORES] = it
    caps = tuple(max(1, max((it[2] for it in row if it is not None), default=1))
                 for row in grid)
    offs = np.zeros(S + 1, dtype=np.int64)
    offs[1:] = np.cumsum(caps)
    T = int(offs[-1])

    # Split slots into two load halves balanced by bytes (half A slightly
    # lighter so the PE starts on it while half B is still streaming), then
    # pack each half's slots into PSUM groups of <= GROUP_COLS columns.
    slot_bytes = [(H + O + int(caps[s])) * 256 for s in range(S)]
    total_b = sum(slot_bytes)
    acc, s_split = 0, S
    for s in range(S):
        acc += slot_bytes[s]
        if acc >= total_b * 0.48:
            s_split = s + 1
            break
    s_split = max(1, min(s_split, S))

    groups = []
    for (lo, hi) in ((0, s_split), (s_split, S)):
        s0 = lo
        while s0 < hi:
            s1 = s0 + 1
            while s1 < hi and int(offs[s1 + 1] - offs[s0]) <= GROUP_COLS:
                s1 += 1
            groups.append((s0, s1))
            s0 = s1
    n_a = sum(1 for (s0, s1) in groups if s1 <= s_split)

    # The blob is loaded as two halves, one DMA per HWDGE ring (SP + ACT),
    # each a [128, W_half] row-major block with its groups side by side as
    # column ranges (columns per group: w1 ns*H | w2 ns*O | x cols).
    # Half A carries the first and last groups (processed 1st and 2nd),
    # half B the middle — both rings stream concurrently and the PE starts
    # on half A while half B is still arriving.
    G = len(groups)
    halves = [list(range(n_a)), list(range(n_a, G))]
    proc = halves[0] + halves[1]            # PE processing order

    def gwidth(gi):
        s0, s1 = groups[gi]
        return (s1 - s0) * (H + O) + int(offs[s1] - offs[s0])

    gbase = {}                              # gi -> (half, col base in half)
    half_w = []
    for hi, gis in enumerate(halves):
        w = 0
        for gi in gis:
            gbase[gi] = (hi, w)
            w += gwidth(gi)
        half_w.append(w)
    Z = 128 * (half_w[0] + half_w[1])

    return {
        "order": order, "starts": starts, "grid": grid,
        "S": S, "caps": caps, "offs": offs, "T": T,
        "groups": groups, "Z": Z, "zero_bias": zero_bias,
        "halves": halves, "proc": proc, "gbase": gbase, "half_w": half_w,
    }


_NC_CACHE: dict = {}


def _build_nc(plan):
    S, caps, T, Z = plan["S"], plan["caps"], plan["T"], plan["Z"]
    zero_bias = plan["zero_bias"]
    key = (S, caps, zero_bias)
    if key in _NC_CACHE:
        return _NC_CACHE[key]

    offs, groups = plan["offs"], plan["groups"]
    gbase, half_w, proc = plan["gbase"], plan["half_w"], plan["proc"]
    G = len(groups)

    nc = bacc.Bacc("TRN2", target_bir_lowering=False, debug=False,
                   enable_partition_id=False)
    blob_d = nc.dram_tensor("blob", [Z], F16, kind="ExternalInput").ap()
    if not zero_bias:
        bias_d = nc.dram_tensor("bias", [128, 2 * S], F32,
                                kind="ExternalInput").ap()
    out_d = nc.dram_tensor("outT", [O * T], F16, kind="ExternalOutput").ap()

    # Two halves, one DMA per HWDGE ring (SP + ACT): the SDMA engines drain
    # both rings' packets concurrently, so the two big transfers (3-4KB
    # per-partition lines) sustain ~2x the single-ring rate; fine-grained
    # staggered loads can't beat this because SDMA round-robin is
    # packet-fair and equalizes completion times anyway.
    with tile.TileContext(nc) as tc, ExitStack() as ctx:
        loads = ctx.enter_context(tc.tile_pool(name="loads", bufs=1))
        hbuf = ctx.enter_context(tc.tile_pool(name="hbuf", bufs=3))
        obuf = ctx.enter_context(tc.tile_pool(name="obuf", bufs=3))
        ps1p = ctx.enter_context(tc.tile_pool(name="ps1p", bufs=2, space="PSUM"))
        ps2p = ctx.enter_context(tc.tile_pool(name="ps2p", bufs=2, space="PSUM"))

        WA, WB = half_w[0], half_w[1]
        blk_a = loads.tile([128, WA], F16)
        nc.sync.dma_start(out=blk_a,
                          in_=blob_d[0:128 * WA].rearrange("(p w) -> p w", p=128))
        if WB:
            blk_b = loads.tile([128, WB], F16)
            nc.scalar.dma_start(
                out=blk_b,
                in_=blob_d[128 * WA:Z].rearrange("(p w) -> p w", p=128))

        def group_view(gi):
            hi, base = gbase[gi]
            return (blk_a if hi == 0 else blk_b), base

        # PE is idle for ~4us while the blob streams in; HAM keeps a cold PE
        # at 1.2GHz until it has seen ~3.4us of sustained activity. Burn the
        # DMA wait with dummy matmuls on a zeroed tile so the real matmuls
        # run warm at 2.4GHz. Alternating two PSUM banks keeps them
        # back-to-back (~427ns each); they must drain before the first load
        # lands so the real stream is never queued behind them.
        warm = ctx.enter_context(tc.tile_pool(name="warm", bufs=1))
        wz = warm.tile([128, 512], F16)
        nc.vector.memset(wz, 0.0)
        wps = ctx.enter_context(
            tc.tile_pool(name="wps", bufs=1, space="PSUM"))
        wp0 = wps.tile([128, 512], F32, name="wp0")
        wp1 = wps.tile([128, 512], F32, name="wp1")
        for j in range(7):
            nc.tensor.matmul(wp0 if j % 2 == 0 else wp1,
                             lhsT=wz[:, 0:128], rhs=wz,
                             start=True, stop=True)
        if not zero_bias:
            consts = ctx.enter_context(tc.tile_pool(name="consts", bufs=1))
            bias = consts.tile([128, 2 * S], F32)
            nc.sync.dma_start(out=bias, in_=bias_d)

        state = {}      # per live group: tiles needed by the layer-2 phase

        def phase1(gi):
            s0, s1 = groups[gi]
            ns = s1 - s0
            co0, co1 = int(offs[s0]), int(offs[s1])
            cols = co1 - co0
            blk, base = group_view(gi)
            xv_base = base + ns * (H + O)
            ps1 = ps1p.tile([H, cols], F32, tag="ps1", name=f"ps1_{gi}")
            for s in range(s0, s1):
                i, B = s - s0, int(caps[s])
                lo = int(offs[s]) - co0
                nc.tensor.matmul(ps1[:, lo:lo + B],
                                 lhsT=blk[:, base + i * H:base + (i + 1) * H],
                                 rhs=blk[:, xv_base + lo:xv_base + lo + B],
                                 start=True, stop=True)
            h_g = hbuf.tile([H, cols], F16, tag="h", name=f"h_{gi}")
            if zero_bias:
                nc.vector.tensor_scalar_max(h_g, ps1, 0.0)
            else:
                for s in range(s0, s1):
                    i, B = s - s0, int(caps[s])
                    lo = int(offs[s]) - co0
                    nc.vector.tensor_scalar(
                        h_g[:, lo:lo + B], ps1[:, lo:lo + B], bias[:, s:s + 1],
                        0.0, mybir.AluOpType.add, mybir.AluOpType.max)
            state[gi] = h_g

        def phase2(gi):
            s0, s1 = groups[gi]
            ns = s1 - s0
            co0, co1 = int(offs[s0]), int(offs[s1])
            cols = co1 - co0
            h_g = state.pop(gi)
            blk, base = group_view(gi)
            w2_base = base + ns * H
            ps2 = ps2p.tile([O, cols], F32, tag="ps2", name=f"ps2_{gi}")
            for s in range(s0, s1):
                i, B = s - s0, int(caps[s])
                lo = int(offs[s]) - co0
                nc.tensor.matmul(ps2[:, lo:lo + B],
                                 lhsT=blk[:, w2_base + i * O:w2_base + (i + 1) * O],
                                 rhs=h_g[:, lo:lo + B], start=True, stop=True)
            o_g = obuf.tile([O, cols], F16, tag="o", name=f"o_{gi}")
            if zero_bias:
                nc.vector.tensor_copy(o_g, ps2)
            else:
                for s in range(s0, s1):
                    i, B = s - s0, int(caps[s])
                    lo = int(offs[s]) - co0
                    nc.vector.tensor_scalar_add(o_g[:, lo:lo + B],
                                                ps2[:, lo:lo + B],
                                                bias[0:O, S + s:S + s + 1])
            dst = out_d[O * co0:O * co1].rearrange("(p w) -> p w", p=O)
            nc.scalar.dma_start(out=dst, in_=o_g)

        # software pipeline: layer-2 of group g rides behind layer-1 of the
        # next group in processing order
        phase1(proc[0])
        for i in range(1, G):
            phase1(proc[i])
            phase2(proc[i - 1])
        phase2(proc[G - 1])

    nc.compile()
    _NC_CACHE[key] = nc
    return nc


def _shard_inputs(x, W1, b1, W2, b2, plan):
    S, offs, Z = plan["S"], plan["offs"], plan["Z"]
    order, starts, grid = plan["order"], plan["starts"], plan["grid"]
    groups = plan["groups"]
    gbase, half_w = plan["gbase"], plan["half_w"]

    x16 = x.astype(np.float16)
    W116 = W1.astype(np.float16)
    W216 = W2.astype(np.float16)

    split = 128 * half_w[0]
    in_maps = []
    for k in range(NCORES):
        blob = np.zeros(Z, dtype=np.float16)
        blks = [blob[0:split].reshape(128, half_w[0]),
                blob[split:Z].reshape(128, half_w[1]) if half_w[1] else None]
        if not plan["zero_bias"]:
            biasc = np.zeros((128, 2 * S), dtype=np.float32)
        for gi, (s0, s1) in enumerate(groups):
            ns = s1 - s0
            co0 = int(offs[s0])
            cols = int(offs[s1]) - co0
            W_g = ns * (H + O) + cols
            hi, base = gbase[gi]
            gb = blks[hi][:, base:base + W_g]
            for s in range(s0, s1):
                it = grid[s][k]
                if it is None:
                    continue
                i = s - s0
                c, o, ln = it
                toks = order[starts[c] + o: starts[c] + o + ln]
                gb[:, i * H:(i + 1) * H] = W116[c]
                gb[:, ns * H + i * O:ns * H + (i + 1) * O] = W216[c]
                xoff = ns * (H + O) + (int(offs[s]) - co0)
                gb[:, xoff:xoff + ln] = x16[toks].T
                if not plan["zero_bias"]:
                    biasc[:, s] = b1[c]
                    biasc[0:O, S + s] = b2[c]
        m = {"blob": blob}
        if not plan["zero_bias"]:
            m["bias"] = biasc
        in_maps.append(m)
    return in_maps


def _unshard(results, plan):
    S, offs, T = plan["S"], plan["offs"], plan["T"]
    order, starts, grid = plan["order"], plan["starts"], plan["grid"]
    groups = plan["groups"]
    out = np.empty((N, O), dtype=np.float32)
    for k in range(NCORES):
        flat = results[k]["outT"].astype(np.float32)
        for (s0, s1) in groups:
            co0, co1 = int(offs[s0]), int(offs[s1])
            blk = flat[O * co0:O * co1].reshape(O, co1 - co0)
            for s in range(s0, s1):
                it = grid[s][k]
                if it is None:
                    continue
                c, o, ln = it
                toks = order[starts[c] + o: starts[c] + o + ln]
                lo = int(offs[s]) - co0
                out[toks] = blk[:, lo:lo + ln].T
    return out


def _execute(x, cat_ids, W1, b1, W2, b2, trace=False):
    x = np.asarray(x, dtype=np.float32)
    W1 = np.asarray(W1, dtype=np.float32)
    b1 = np.asarray(b1, dtype=np.float32)
    W2 = np.asarray(W2, dtype=np.float32)
    b2 = np.asarray(b2, dtype=np.float32)

    zero_bias = not (b1.any() or b2.any())
    plan = _plan(cat_ids, zero_bias)
    nc = _build_nc(plan)
    in_maps = _shard_inputs(x, W1, b1, W2, b2, plan)
    res = run_bass_kernel_spmd(nc, in_maps, list(range(NCORES)), trace=trace)
    out = _unshard(res.results, plan)
    return out, res


def kernel(x, cat_ids, W1, b1, W2, b2):
    out, _ = _execute(x, cat_ids, W1, b1, W2, b2, trace=False)
    return out


# revision 57
# speedup vs baseline: 1.0549x; 1.0235x over previous
"""Category-specific MLP (MoE-style routing) for Trainium2, 8 NeuronCores.

Reference computation (per token n):
    h   = relu(x[n] @ W1[cat[n]] + b1[cat[n]])      x:[N,128]  W1:[100,128,128]
    out = h @ W2[cat[n]] + b2[cat[n]]               W2:[100,128,64]

Strategy (expert-parallel, MoE-style):
  * Host: sort tokens by category. Split any category with more than 512
    tokens into work items of <=512 tokens. Sort items by size (desc) and
    assign item of rank r to (core r%8, slot r//8). All cores run the same
    SPMD program with S slots; slot s has fixed column capacity caps[s] =
    size of the largest item in that slot across cores, so the instruction
    stream and shapes are identical on every core while padding stays
    minimal (~5% for the target distribution).
  * Everything is kept feature-on-partitions (transposed). Slots are packed
    into PSUM groups of <=512 total columns (one PSUM bank per group-layer).
    The per-core fp16 blob holds two row-major [128, W] halves (groups side
    by side, per group: w1 ns*128 | w2 ns*64 | xT cols); each half is one
    big-line DMA on its own HWDGE ring (SP / ACT) so both stream
    concurrently at ~2x single-ring rate.
    Per slot (fp16 matmuls, fp32 PSUM accumulate):
        psum1[:, lo:lo+B] = W1_s^T @ xT_s        (PE)
        psum2[:, lo:lo+B] = W2_s^T @ h_s         (PE)
    Per group (merged PSUM evacuation on DVE, valid because b1/b2 == 0;
    a per-slot bias path is emitted instead when biases are nonzero):
        h_g   = max(psum1_g, 0)   -> fp16 SBUF
        out_g = copy(psum2_g)     -> fp32 SBUF -> DMA
    Groups are software-pipelined (layer-2 of group g emitted after layer-1
    of group g+1) so the PE stream never waits on an evacuation.
  * Host: scatter outT columns back to the original token order.

fp16 numerics: inputs are rounded to fp16 (10-bit mantissa), accumulation
is fp32 in PSUM. Measured vs the fp32 reference: resid_var ~2e-7,
absmax-relative error ~5e-4.
"""

from contextlib import ExitStack

import numpy as np

import concourse.bass as bass
import concourse.mybir as mybir
import concourse.tile as tile
from concourse import bacc
from concourse.bass_utils import run_bass_kernel_spmd

N, C, D, H, O = 8192, 100, 128, 128, 64
NCORES = 8
MAX_ITEM = 512      # PSUM bank / moving-operand limit (fp32 columns)
GROUP_COLS = 512    # column budget per group (one PSUM bank, fp32)

F16 = mybir.dt.float16
F32 = mybir.dt.float32


def _plan(cat_ids: np.ndarray, zero_bias: bool):
    """Host-side routing plan: work items -> (core, slot) assignment."""
    cat_ids = np.asarray(cat_ids).astype(np.int64)
    counts = np.bincount(cat_ids, minlength=C)
    NC = len(counts)                                    # robust to ids >= C
    order = np.argsort(cat_ids, kind="stable")          # token ids sorted by cat
    starts = np.zeros(NC, dtype=np.int64)
    starts[1:] = np.cumsum(counts)[:-1]

    items = []                                          # (cat, start_in_cat, len)
    for c in range(NC):
        cnt = int(counts[c])
        o = 0
        while o < cnt:
            ln = min(MAX_ITEM, cnt - o)
            items.append((c, o, ln))
            o += ln
    items.sort(key=lambda it: -it[2])

    S = (len(items) + NCORES - 1) // NCORES
    grid = [[None] * NCORES for _ in range(S)]          # grid[s][k] = item|None
    for r, it in enumerate(items):
        grid[r // NCORES][r % NCORES] = it
    caps = tuple(max(1, max((it[2] for it in row if it is not None), default=1))
                 for row in grid)
    offs = np.zeros(S + 1, dtype=np.int64)
    offs[1:] = np.cumsum(caps)
    T = int(offs[-1])

    # Split slots into two load halves balanced by bytes (half A slightly
    # lighter so the PE starts on it while half B is still streaming), then
    # pack each half's slots into PSUM groups of <= GROUP_COLS columns.
    slot_bytes = [(H + O + int(caps[s])) * 256 for s in range(S)]
    total_b = sum(slot_bytes)
    acc, s_split = 0, S
    for s in range(S):
        acc += slot_bytes[s]
        if acc >= total_b * 0.39:
            s_split = s + 1
            break
    s_split = max(1, min(s_split, S))

    groups = []
    for (lo, hi) in ((0, s_split), (s_split, S)):
        s0 = lo
        while s0 < hi:
            s1 = s0 + 1
            while s1 < hi and int(offs[s1 + 1] - offs[s0]) <= GROUP_COLS:
                s1 += 1
            groups.append((s0, s1))
            s0 = s1
    n_a = sum(1 for (s0, s1) in groups if s1 <= s_split)

    # The blob is loaded as two halves, one DMA per HWDGE ring (SP + ACT),
    # each a [128, W_half] row-major block with its groups side by side as
    # column ranges (columns per group: w1 ns*H | w2 ns*O | x cols).
    # Half A carries the first and last groups (processed 1st and 2nd),
    # half B the middle — both rings stream concurrently and the PE starts
    # on half A while half B is still arriving.
    G = len(groups)
    halves = [list(range(n_a)), list(range(n_a, G))]
    proc = halves[0] + halves[1]            # PE processing order

    def gwidth(gi):
        s0, s1 = groups[gi]
        return (s1 - s0) * (H + O) + int(offs[s1] - offs[s0])

    gbase = {}                              # gi -> (half, col base in half)
    half_w = []
    for hi, gis in enumerate(halves):
        w = 0
        for gi in gis:
            gbase[gi] = (hi, w)
            w += gwidth(gi)
        half_w.append(w)
    Z = 128 * (half_w[0] + half_w[1])

    return {
        "order": order, "starts": starts, "grid": grid,
        "S": S, "caps": caps, "offs": offs, "T": T,
        "groups": groups, "Z": Z, "zero_bias": zero_bias,
        "halves": halves, "proc": proc, "gbase": gbase, "half_w": half_w,
    }


_NC_CACHE: dict = {}


def _build_nc(plan):
    S, caps, T, Z = plan["S"], plan["caps"], plan["T"], plan["Z"]
    zero_bias = plan["zero_bias"]
    key = (S, caps, zero_bias)
    if key in _NC_CACHE:
        return _NC_CACHE[key]

    offs, groups = plan["offs"], plan["groups"]
    gbase, half_w, proc = plan["gbase"], plan["half_w"], plan["proc"]
    G = len(groups)

    nc = bacc.Bacc("TRN2", target_bir_lowering=False, debug=False,
                   enable_partition_id=False)
    blob_d = nc.dram_tensor("blob", [Z], F16, kind="ExternalInput").ap()
    if not zero_bias:
        bias_d = nc.dram_tensor("bias", [128, 2 * S], F32,
                                kind="ExternalInput").ap()
    out_d = nc.dram_tensor("outT", [O * T], F16, kind="ExternalOutput").ap()

    # Two halves, one DMA per HWDGE ring (SP + ACT): the SDMA engines drain
    # both rings' packets concurrently, so the two big transfers (3-4KB
    # per-partition lines) sustain ~2x the single-ring rate; fine-grained
    # staggered loads can't beat this because SDMA round-robin is
    # packet-fair and equalizes completion times anyway.
    with tile.TileContext(nc) as tc, ExitStack() as ctx:
        loads = ctx.enter_context(tc.tile_pool(name="loads", bufs=1))
        hbuf = ctx.enter_context(tc.tile_pool(name="hbuf", bufs=3))
        obuf = ctx.enter_context(tc.tile_pool(name="obuf", bufs=3))
        ps1p = ctx.enter_context(tc.tile_pool(name="ps1p", bufs=2, space="PSUM"))
        ps2p = ctx.enter_context(tc.tile_pool(name="ps2p", bufs=2, space="PSUM"))

        WA, WB = half_w[0], half_w[1]
        blk_a = loads.tile([128, WA], F16)
        nc.sync.dma_start(out=blk_a,
                          in_=blob_d[0:128 * WA].rearrange("(p w) -> p w", p=128))
        if WB:
            blk_b = loads.tile([128, WB], F16)
            nc.scalar.dma_start(
                out=blk_b,
                in_=blob_d[128 * WA:Z].rearrange("(p w) -> p w", p=128))

        def group_view(gi):
            hi, base = gbase[gi]
            return (blk_a if hi == 0 else blk_b), base

        # PE is idle for ~4us while the blob streams in; HAM keeps a cold PE
        # at 1.2GHz until it has seen ~3.4us of sustained activity. Burn the
        # DMA wait with dummy matmuls on a zeroed tile so the real matmuls
        # run warm at 2.4GHz. Alternating two PSUM banks keeps them
        # back-to-back (~427ns each); they must drain before the first load
        # lands so the real stream is never queued behind them.
        warm = ctx.enter_context(tc.tile_pool(name="warm", bufs=1))
        wz = warm.tile([128, 512], F16)
        nc.vector.memset(wz, 0.0)
        wps = ctx.enter_context(
            tc.tile_pool(name="wps", bufs=1, space="PSUM"))
        wp0 = wps.tile([128, 512], F32, name="wp0")
        wp1 = wps.tile([128, 512], F32, name="wp1")
        for j in range(7):
            nc.tensor.matmul(wp0 if j % 2 == 0 else wp1,
                             lhsT=wz[:, 0:128], rhs=wz,
                             start=True, stop=True)
        if not zero_bias:
            consts = ctx.enter_context(tc.tile_pool(name="consts", bufs=1))
            bias = consts.tile([128, 2 * S], F32)
            nc.sync.dma_start(out=bias, in_=bias_d)

        state = {}      # per live group: tiles needed by the layer-2 phase

        def phase1(gi):
            s0, s1 = groups[gi]
            ns = s1 - s0
            co0, co1 = int(offs[s0]), int(offs[s1])
            cols = co1 - co0
            blk, base = group_view(gi)
            xv_base = base + ns * (H + O)
            ps1 = ps1p.tile([H, cols], F32, tag="ps1", name=f"ps1_{gi}")
            for s in range(s0, s1):
                i, B = s - s0, int(caps[s])
                lo = int(offs[s]) - co0
                nc.tensor.matmul(ps1[:, lo:lo + B],
                                 lhsT=blk[:, base + i * H:base + (i + 1) * H],
                                 rhs=blk[:, xv_base + lo:xv_base + lo + B],
                                 start=True, stop=True)
            h_g = hbuf.tile([H, cols], F16, tag="h", name=f"h_{gi}")
            if zero_bias:
                nc.vector.tensor_scalar_max(h_g, ps1, 0.0)
            else:
                for s in range(s0, s1):
                    i, B = s - s0, int(caps[s])
                    lo = int(offs[s]) - co0
                    nc.vector.tensor_scalar(
                        h_g[:, lo:lo + B], ps1[:, lo:lo + B], bias[:, s:s + 1],
                        0.0, mybir.AluOpType.add, mybir.AluOpType.max)
            state[gi] = h_g

        def phase2(gi):
            s0, s1 = groups[gi]
            ns = s1 - s0
            co0, co1 = int(offs[s0]), int(offs[s1])
            cols = co1 - co0
            h_g = state.pop(gi)
            blk, base = group_view(gi)
            w2_base = base + ns * H
            ps2 = ps2p.tile([O, cols], F32, tag="ps2", name=f"ps2_{gi}")
            for s in range(s0, s1):
                i, B = s - s0, int(caps[s])
                lo = int(offs[s]) - co0
                nc.tensor.matmul(ps2[:, lo:lo + B],
                                 lhsT=blk[:, w2_base + i * O:w2_base + (i + 1) * O],
                                 rhs=h_g[:, lo:lo + B], start=True, stop=True)
            o_g = obuf.tile([O, cols], F16, tag="o", name=f"o_{gi}")
            if zero_bias:
                nc.vector.tensor_copy(o_g, ps2)
            else:
                for s in range(s0, s1):
                    i, B = s - s0, int(caps[s])
                    lo = int(offs[s]) - co0
                    nc.vector.tensor_scalar_add(o_g[:, lo:lo + B],
                                                ps2[:, lo:lo + B],
                                                bias[0:O, S + s:S + s + 1])
            dst = out_d[O * co0:O * co1].rearrange("(p w) -> p w", p=O)
            nc.scalar.dma_start(out=dst, in_=o_g)

        # software pipeline: layer-2 of group g rides behind layer-1 of the
        # next group in processing order
        phase1(proc[0])
        for i in range(1, G):
            phase1(proc[i])
            phase2(proc[i - 1])
        phase2(proc[G - 1])

    nc.compile()
    _NC_CACHE[key] = nc
    return nc


def _shard_inputs(x, W1, b1, W2, b2, plan):
    S, offs, Z = plan["S"], plan["offs"], plan["Z"]
    order, starts, grid = plan["order"], plan["starts"], plan["grid"]
    groups = plan["groups"]
    gbase, half_w = plan["gbase"], plan["half_w"]

    x16 = x.astype(np.float16)
    W116 = W1.astype(np.float16)
    W216 = W2.astype(np.float16)

    split = 128 * half_w[0]
    in_maps = []
    for k in range(NCORES):
        blob = np.zeros(Z, dtype=np.float16)
        blks = [blob[0:split].reshape(128, half_w[0]),
                blob[split:Z].reshape(128, half_w[1]) if half_w[1] else None]
        if not plan["zero_bias"]:
            biasc = np.zeros((128, 2 * S), dtype=np.float32)
        for gi, (s0, s1) in enumerate(groups):
            ns = s1 - s0
            co0 = int(offs[s0])
            cols = int(offs[s1]) - co0
            W_g = ns * (H + O) + cols
            hi, base = gbase[gi]
            gb = blks[hi][:, base:base + W_g]
            for s in range(s0, s1):
                it = grid[s][k]
                if it is None:
                    continue
                i = s - s0
                c, o, ln = it
                toks = order[starts[c] + o: starts[c] + o + ln]
                gb[:, i * H:(i + 1) * H] = W116[c]
                gb[:, ns * H + i * O:ns * H + (i + 1) * O] = W216[c]
                xoff = ns * (H + O) + (int(offs[s]) - co0)
                gb[:, xoff:xoff + ln] = x16[toks].T
                if not plan["zero_bias"]:
                    biasc[:, s] = b1[c]
                    biasc[0:O, S + s] = b2[c]
        m = {"blob": blob}
        if not plan["zero_bias"]:
            m["bias"] = biasc
        in_maps.append(m)
    return in_maps


def _unshard(results, plan):
    S, offs, T = plan["S"], plan["offs"], plan["T"]
    order, starts, grid = plan["order"], plan["starts"], plan["grid"]
    groups = plan["groups"]
    out = np.empty((N, O), dtype=np.float32)
    for k in range(NCORES):
        flat = results[k]["outT"].astype(np.float32)
        for (s0, s1) in groups:
            co0, co1 = int(offs[s0]), int(offs[s1])
            blk = flat[O * co0:O * co1].reshape(O, co1 - co0)
            for s in range(s0, s1):
                it = grid[s][k]
                if it is None:
                    continue
                c, o, ln = it
                toks = order[starts[c] + o: starts[c] + o + ln]
                lo = int(offs[s]) - co0
                out[toks] = blk[:, lo:lo + ln].T
    return out


def _execute(x, cat_ids, W1, b1, W2, b2, trace=False):
    x = np.asarray(x, dtype=np.float32)
    W1 = np.asarray(W1, dtype=np.float32)
    b1 = np.asarray(b1, dtype=np.float32)
    W2 = np.asarray(W2, dtype=np.float32)
    b2 = np.asarray(b2, dtype=np.float32)

    zero_bias = not (b1.any() or b2.any())
    plan = _plan(cat_ids, zero_bias)
    nc = _build_nc(plan)
    in_maps = _shard_inputs(x, W1, b1, W2, b2, plan)
    res = run_bass_kernel_spmd(nc, in_maps, list(range(NCORES)), trace=trace)
    out = _unshard(res.results, plan)
    return out, res


def kernel(x, cat_ids, W1, b1, W2, b2):
    out, _ = _execute(x, cat_ids, W1, b1, W2, b2, trace=False)
    return out


# revision 62
# speedup vs baseline: 1.0605x; 1.0054x over previous
"""Category-specific MLP (MoE-style routing) for Trainium2, 8 NeuronCores.

Reference computation (per token n):
    h   = relu(x[n] @ W1[cat[n]] + b1[cat[n]])      x:[N,128]  W1:[100,128,128]
    out = h @ W2[cat[n]] + b2[cat[n]]               W2:[100,128,64]

Strategy (expert-parallel, MoE-style):
  * Host: sort tokens by category. Split any category with more than 512
    tokens into work items of <=512 tokens. Sort items by size (desc) and
    assign item of rank r to (core r%8, slot r//8). All cores run the same
    SPMD program with S slots; slot s has fixed column capacity caps[s] =
    size of the largest item in that slot across cores, so the instruction
    stream and shapes are identical on every core while padding stays
    minimal (~5% for the target distribution).
  * Everything is kept feature-on-partitions (transposed). Slots are packed
    into PSUM groups of <=512 total columns (one PSUM bank per group-layer).
    The per-core fp16 blob holds two row-major [128, W] halves (groups side
    by side, per group: w1 ns*128 | w2 ns*64 | xT cols); each half is one
    big-line DMA on its own HWDGE ring (SP / ACT) so both stream
    concurrently at ~2x single-ring rate.
    Per slot (fp16 matmuls, fp32 PSUM accumulate):
        psum1[:, lo:lo+B] = W1_s^T @ xT_s        (PE)
        psum2[:, lo:lo+B] = W2_s^T @ h_s         (PE)
    Per group (merged PSUM evacuation on DVE, valid because b1/b2 == 0;
    a per-slot bias path is emitted instead when biases are nonzero):
        h_g   = max(psum1_g, 0)   -> fp16 SBUF
        out_g = copy(psum2_g)     -> fp32 SBUF -> DMA
    Groups are software-pipelined (layer-2 of group g emitted after layer-1
    of group g+1) so the PE stream never waits on an evacuation.
  * Host: scatter outT columns back to the original token order.

fp16 numerics: inputs are rounded to fp16 (10-bit mantissa), accumulation
is fp32 in PSUM. Measured vs the fp32 reference: resid_var ~2e-7,
absmax-relative error ~5e-4.
"""

from contextlib import ExitStack

import numpy as np

import concourse.bass as bass
import concourse.mybir as mybir
import concourse.tile as tile
from concourse import bacc
from concourse.bass_utils import run_bass_kernel_spmd

N, C, D, H, O = 8192, 100, 128, 128, 64
NCORES = 8
MAX_ITEM = 512      # PSUM bank / moving-operand limit (fp32 columns)
GROUP_COLS = 512    # column budget per group (one PSUM bank, fp32)

F16 = mybir.dt.float16
F32 = mybir.dt.float32


def _plan(cat_ids: np.ndarray, zero_bias: bool):
    """Host-side routing plan: work items -> (core, slot) assignment."""
    cat_ids = np.asarray(cat_ids).astype(np.int64)
    counts = np.bincount(cat_ids, minlength=C)
    NC = len(counts)                                    # robust to ids >= C
    order = np.argsort(cat_ids, kind="stable")          # token ids sorted by cat
    starts = np.zeros(NC, dtype=np.int64)
    starts[1:] = np.cumsum(counts)[:-1]

    items = []                                          # (cat, start_in_cat, len)
    for c in range(NC):
        cnt = int(counts[c])
        o = 0
        while o < cnt:
            ln = min(MAX_ITEM, cnt - o)
            items.append((c, o, ln))
            o += ln
    items.sort(key=lambda it: -it[2])

    S = (len(items) + NCORES - 1) // NCORES
    grid = [[None] * NCORES for _ in range(S)]          # grid[s][k] = item|None
    for r, it in enumerate(items):
        grid[r // NCORES][r % NCORES] = it
    caps = tuple(max(1, max((it[2] for it in row if it is not None), default=1))
                 for row in grid)
    offs = np.zeros(S + 1, dtype=np.int64)
    offs[1:] = np.cumsum(caps)
    T = int(offs[-1])

    # Split slots into two load halves balanced by bytes (half A slightly
    # lighter so the PE starts on it while half B is still streaming), then
    # pack each half's slots into PSUM groups of <= GROUP_COLS columns.
    slot_bytes = [(H + O + int(caps[s])) * 256 for s in range(S)]
    total_b = sum(slot_bytes)
    acc, s_split = 0, S
    for s in range(S):
        acc += slot_bytes[s]
        if acc >= total_b * 0.39:
            s_split = s + 1
            break
    s_split = max(1, min(s_split, S))

    groups = []
    for (lo, hi) in ((0, s_split), (s_split, S)):
        s0 = lo
        while s0 < hi:
            s1 = s0 + 1
            while s1 < hi and int(offs[s1 + 1] - offs[s0]) <= GROUP_COLS:
                s1 += 1
            groups.append((s0, s1))
            s0 = s1
    n_a = sum(1 for (s0, s1) in groups if s1 <= s_split)

    # The blob is loaded as two halves, one DMA per HWDGE ring (SP + ACT),
    # each a [128, W_half] row-major block with its groups side by side as
    # column ranges (columns per group: w1 ns*H | w2 ns*O | x cols).
    # Half A carries the first and last groups (processed 1st and 2nd),
    # half B the middle — both rings stream concurrently and the PE starts
    # on half A while half B is still arriving.
    G = len(groups)
    halves = [list(range(n_a)), list(range(n_a, G))]
    proc = halves[0] + halves[1]            # PE processing order

    def gwidth(gi):
        s0, s1 = groups[gi]
        return (s1 - s0) * (H + O) + int(offs[s1] - offs[s0])

    # each group is its own contiguous [128, W_g] row-major block in the
    # blob, loaded by its own DMA; half A groups ride the SP HWDGE ring,
    # half B groups the ACT ring (per-ring FIFO staggers completions in
    # processing order while both rings stream concurrently)
    ghalf = {gi: hi for hi, gis in enumerate(halves) for gi in gis}
    bpos = {}                               # gi -> element offset of block
    pos = 0
    for gi in proc:
        bpos[gi] = pos
        pos += 128 * gwidth(gi)
    Z = pos

    return {
        "order": order, "starts": starts, "grid": grid,
        "S": S, "caps": caps, "offs": offs, "T": T,
        "groups": groups, "Z": Z, "zero_bias": zero_bias,
        "halves": halves, "proc": proc, "ghalf": ghalf, "bpos": bpos,
        "gwidth": {gi: gwidth(gi) for gi in range(G)},
    }


_NC_CACHE: dict = {}


def _build_nc(plan):
    S, caps, T, Z = plan["S"], plan["caps"], plan["T"], plan["Z"]
    zero_bias = plan["zero_bias"]
    key = (S, caps, zero_bias)
    if key in _NC_CACHE:
        return _NC_CACHE[key]

    offs, groups = plan["offs"], plan["groups"]
    ghalf, bpos, proc = plan["ghalf"], plan["bpos"], plan["proc"]
    gw = plan["gwidth"]
    G = len(groups)

    nc = bacc.Bacc("TRN2", target_bir_lowering=False, debug=False,
                   enable_partition_id=False)
    blob_d = nc.dram_tensor("blob", [Z], F16, kind="ExternalInput").ap()
    if not zero_bias:
        bias_d = nc.dram_tensor("bias", [128, 2 * S], F32,
                                kind="ExternalInput").ap()
    out_d = nc.dram_tensor("outT", [O * T], F16, kind="ExternalOutput").ap()

    # One DMA per group block, half A on the SP HWDGE ring, half B on the
    # ACT ring: both rings stream concurrently (~2x single-ring rate) and
    # each ring's FIFO staggers its groups' completions in processing
    # order, so the PE picks up each group as it lands.
    with tile.TileContext(nc) as tc, ExitStack() as ctx:
        loads = ctx.enter_context(tc.tile_pool(name="loads", bufs=1))
        hbuf = ctx.enter_context(tc.tile_pool(name="hbuf", bufs=3))
        obuf = ctx.enter_context(tc.tile_pool(name="obuf", bufs=3))
        ps1p = ctx.enter_context(tc.tile_pool(name="ps1p", bufs=2, space="PSUM"))
        ps2p = ctx.enter_context(tc.tile_pool(name="ps2p", bufs=2, space="PSUM"))

        gtiles = {}
        for gi in proc:
            g_sb = loads.tile([128, gw[gi]], F16, tag=f"blk_{gi}",
                              name=f"blk_{gi}")
            eng = nc.sync if ghalf[gi] == 0 else nc.scalar
            eng.dma_start(out=g_sb,
                          in_=blob_d[bpos[gi]:bpos[gi] + 128 * gw[gi]]
                          .rearrange("(p w) -> p w", p=128))
            gtiles[gi] = g_sb

        def group_view(gi):
            return gtiles[gi], 0

        # PE is idle for ~4us while the blob streams in; HAM keeps a cold PE
        # at 1.2GHz until it has seen ~3.4us of sustained activity. Burn the
        # DMA wait with dummy matmuls on a zeroed tile so the real matmuls
        # run warm at 2.4GHz. Alternating two PSUM banks keeps them
        # back-to-back (~427ns each); they must drain before the first load
        # lands so the real stream is never queued behind them.
        warm = ctx.enter_context(tc.tile_pool(name="warm", bufs=1))
        wz = warm.tile([128, 512], F16)
        nc.vector.memset(wz, 0.0)
        wps = ctx.enter_context(
            tc.tile_pool(name="wps", bufs=1, space="PSUM"))
        wp0 = wps.tile([128, 512], F32, name="wp0")
        wp1 = wps.tile([128, 512], F32, name="wp1")
        for j in range(7):
            nc.tensor.matmul(wp0 if j % 2 == 0 else wp1,
                             lhsT=wz[:, 0:128], rhs=wz,
                             start=True, stop=True)
        if not zero_bias:
            consts = ctx.enter_context(tc.tile_pool(name="consts", bufs=1))
            bias = consts.tile([128, 2 * S], F32)
            nc.sync.dma_start(out=bias, in_=bias_d)

        state = {}      # per live group: tiles needed by the layer-2 phase

        def phase1(gi):
            s0, s1 = groups[gi]
            ns = s1 - s0
            co0, co1 = int(offs[s0]), int(offs[s1])
            cols = co1 - co0
            blk, base = group_view(gi)
            xv_base = base + ns * (H + O)
            ps1 = ps1p.tile([H, cols], F32, tag="ps1", name=f"ps1_{gi}")
            for s in range(s0, s1):
                i, B = s - s0, int(caps[s])
                lo = int(offs[s]) - co0
                nc.tensor.matmul(ps1[:, lo:lo + B],
                                 lhsT=blk[:, base + i * H:base + (i + 1) * H],
                                 rhs=blk[:, xv_base + lo:xv_base + lo + B],
                                 start=True, stop=True)
            h_g = hbuf.tile([H, cols], F16, tag="h", name=f"h_{gi}")
            if zero_bias:
                nc.vector.tensor_scalar_max(h_g, ps1, 0.0)
            else:
                for s in range(s0, s1):
                    i, B = s - s0, int(caps[s])
                    lo = int(offs[s]) - co0
                    nc.vector.tensor_scalar(
                        h_g[:, lo:lo + B], ps1[:, lo:lo + B], bias[:, s:s + 1],
                        0.0, mybir.AluOpType.add, mybir.AluOpType.max)
            state[gi] = h_g

        def phase2(gi):
            s0, s1 = groups[gi]
            ns = s1 - s0
            co0, co1 = int(offs[s0]), int(offs[s1])
            cols = co1 - co0
            h_g = state.pop(gi)
            blk, base = group_view(gi)
            w2_base = base + ns * H
            ps2 = ps2p.tile([O, cols], F32, tag="ps2", name=f"ps2_{gi}")
            for s in range(s0, s1):
                i, B = s - s0, int(caps[s])
                lo = int(offs[s]) - co0
                nc.tensor.matmul(ps2[:, lo:lo + B],
                                 lhsT=blk[:, w2_base + i * O:w2_base + (i + 1) * O],
                                 rhs=h_g[:, lo:lo + B], start=True, stop=True)
            o_g = obuf.tile([O, cols], F16, tag="o", name=f"o_{gi}")
            if zero_bias:
                nc.vector.tensor_copy(o_g, ps2)
            else:
                for s in range(s0, s1):
                    i, B = s - s0, int(caps[s])
                    lo = int(offs[s]) - co0
                    nc.vector.tensor_scalar_add(o_g[:, lo:lo + B],
                                                ps2[:, lo:lo + B],
                                                bias[0:O, S + s:S + s + 1])
            dst = out_d[O * co0:O * co1].rearrange("(p w) -> p w", p=O)
            # the last group's store rides the idle SP ring so it doesn't
            # queue behind the earlier stores on the ACT ring
            st_eng = nc.sync if gi == proc[-1] else nc.scalar
            st_eng.dma_start(out=dst, in_=o_g)

        # software pipeline: layer-2 of group g rides behind layer-1 of the
        # next group in processing order
        phase1(proc[0])
        for i in range(1, G):
            phase1(proc[i])
            phase2(proc[i - 1])
        phase2(proc[G - 1])

    nc.compile()
    _NC_CACHE[key] = nc
    return nc


def _shard_inputs(x, W1, b1, W2, b2, plan):
    S, offs, Z = plan["S"], plan["offs"], plan["Z"]
    order, starts, grid = plan["order"], plan["starts"], plan["grid"]
    groups, bpos = plan["groups"], plan["bpos"]

    x16 = x.astype(np.float16)
    W116 = W1.astype(np.float16)
    W216 = W2.astype(np.float16)

    in_maps = []
    for k in range(NCORES):
        blob = np.zeros(Z, dtype=np.float16)
        if not plan["zero_bias"]:
            biasc = np.zeros((128, 2 * S), dtype=np.float32)
        for gi, (s0, s1) in enumerate(groups):
            ns = s1 - s0
            co0 = int(offs[s0])
            cols = int(offs[s1]) - co0
            W_g = ns * (H + O) + cols
            gb = blob[bpos[gi]:bpos[gi] + 128 * W_g].reshape(128, W_g)
            for s in range(s0, s1):
                it = grid[s][k]
                if it is None:
                    continue
                i = s - s0
                c, o, ln = it
                toks = order[starts[c] + o: starts[c] + o + ln]
                gb[:, i * H:(i + 1) * H] = W116[c]
                gb[:, ns * H + i * O:ns * H + (i + 1) * O] = W216[c]
                xoff = ns * (H + O) + (int(offs[s]) - co0)
                gb[:, xoff:xoff + ln] = x16[toks].T
                if not plan["zero_bias"]:
                    biasc[:, s] = b1[c]
                    biasc[0:O, S + s] = b2[c]
        m = {"blob": blob}
        if not plan["zero_bias"]:
            m["bias"] = biasc
        in_maps.append(m)
    return in_maps


def _unshard(results, plan):
    S, offs, T = plan["S"], plan["offs"], plan["T"]
    order, starts, grid = plan["order"], plan["starts"], plan["grid"]
    groups = plan["groups"]
    out = np.empty((N, O), dtype=np.float32)
    for k in range(NCORES):
        flat = results[k]["outT"].astype(np.float32)
        for (s0, s1) in groups:
            co0, co1 = int(offs[s0]), int(offs[s1])
            blk = flat[O * co0:O * co1].reshape(O, co1 - co0)
            for s in range(s0, s1):
                it = grid[s][k]
                if it is None:
                    continue
                c, o, ln = it
                toks = order[starts[c] + o: starts[c] + o + ln]
                lo = int(offs[s]) - co0
                out[toks] = blk[:, lo:lo + ln].T
    return out


def _execute(x, cat_ids, W1, b1, W2, b2, trace=False):
    x = np.asarray(x, dtype=np.float32)
    W1 = np.asarray(W1, dtype=np.float32)
    b1 = np.asarray(b1, dtype=np.float32)
    W2 = np.asarray(W2, dtype=np.float32)
    b2 = np.asarray(b2, dtype=np.float32)

    zero_bias = not (b1.any() or b2.any())
    plan = _plan(cat_ids, zero_bias)
    nc = _build_nc(plan)
    in_maps = _shard_inputs(x, W1, b1, W2, b2, plan)
    res = run_bass_kernel_spmd(nc, in_maps, list(range(NCORES)), trace=trace)
    out = _unshard(res.results, plan)
    return out, res


def kernel(x, cat_ids, W1, b1, W2, b2):
    out, _ = _execute(x, cat_ids, W1, b1, W2, b2, trace=False)
    return out
